# revision 11
# baseline (speedup 1.0000x reference)
"""Trainium2 Bass kernel for nn_CINLayer.

Computes, for B=2048, C=64, N=64, D=64, F=128:
    cin_out[b,f,d] = sum_{c,n} W[f,c,n] * xj[b,c,d] * x0[b,n,d]   (B, F, D)
    cin_p_out[b,f] = sum_d cin_out[b,f,d]                          (B, F)

Strategy (per NeuronCore, data-parallel over b across 8 cores):
  The einsum is reassociated as one accumulated matmul over K=(c,n)=4096:
     cin_out[f, (b,d)] = W_flat[f, (c,n)] @ H[(c,n), (b,d)]
  with H[(c,n),(b,d)] = xj[b,c,d]*x0[b,n,d] built on the Vector engine in
  bf16 (2x packed mode) one 128-row k-tile at a time.

  A k-tile (ci, ni) covers c in [8ci, 8ci+8) x n in [16ni, 16ni+16), so the
  partition-broadcast operands factorize: H(ci,ni) = xj_rep(ci) * x0_rep(ni),
  and only 8 + 4 = 12 replicated (128, 2048) tiles are DMA'd per batch group
  of 32 rows instead of 32 tiles (each xj_rep is reused for 4 k-tiles, each
  x0_rep for 8) — the minimum replication traffic for materialized operands.
  The replication DMA sources are host-pre-transposed (C,B,D)/(N,B,D) bf16
  copies so every replicated partition row is one contiguous 4KB read.
  The 32 k-tile matmuls (bf16, full PE rate, Nf=512 into 4 PSUM banks)
  accumulate in PSUM; ScalarE evacuates to SBUF; cin_p comes from a
  free-dim tensor_reduce of each PSUM bank.

  Measured on hardware: 376.6 us end-to-end (8 cores), rel-l2 err 3.3e-3.
  Bottleneck: the 8 load-side SDMA engines moving the ~46MB/core of
  replicated operands (~17.5 GB/s each); PE matmul stream ~276 us,
  DVE Hadamard ~313 us, all overlapped.
"""

import sys

for _p in ("/opt/trn_rl_repo", "/root/.axon_site/_ro/trn_rl_repo"):
    if _p not in sys.path:
        sys.path.insert(0, _p)

import numpy as np
import ml_dtypes

BF16 = ml_dtypes.bfloat16

B, C, N, D, F = 2048, 64, 64, 64, 128
N_CORES = 8
B_CORE = B // N_CORES          # 256 batch rows per core
BG = 32                        # batch rows per group (=> 2048 matmul columns)
N_GROUPS = B_CORE // BG        # 16 groups
KT = 32                        # number of 128-row k-tiles (C*N/128)
COLS = BG * D                  # 1024 columns per group
QTR = COLS // 4                # 512 = one PSUM bank of fp32
CA = 8                         # c's per k-tile
NB = 128 // CA                 # n's per k-tile (16)
NCI = C // CA                  # 8 ci tiles per group
NNI = N // NB                  # 4 ni tiles per group

_compiled = None
LAST_RESULTS = None


def _build():
    import concourse.bass as bass
    import concourse.mybir as mybir
    import concourse.tile as tile
    from concourse import bacc

    fp32 = mybir.dt.float32
    bf16 = mybir.dt.bfloat16

    nc = bacc.Bacc("TRN2", target_bir_lowering=False, debug=False)

    xj_r = nc.dram_tensor("xj_r", [C, B_CORE, D], bf16, kind="ExternalInput").ap()
    x0_r = nc.dram_tensor("x0_r", [N, B_CORE, D], bf16, kind="ExternalInput").ap()
    w_t = nc.dram_tensor("w_t", [128, KT, F], bf16, kind="ExternalInput").ap()
    cout = nc.dram_tensor("cout", [B_CORE, F, D], fp32, kind="ExternalOutput").ap()
    cp_t = nc.dram_tensor("cp_t", [F, B_CORE], fp32, kind="ExternalOutput").ap()

    with tile.TileContext(nc) as tc:
        with (
            tc.tile_pool(name="w", bufs=1) as w_pool,
            tc.tile_pool(name="xjrep", bufs=16) as xj_pool,
            tc.tile_pool(name="x0rep", bufs=8) as x0_pool,
            tc.tile_pool(name="h", bufs=8) as h_pool,
            tc.tile_pool(name="ps", bufs=2, space="PSUM") as ps_pool,
            tc.tile_pool(name="co", bufs=2) as co_pool,
            tc.tile_pool(name="cp", bufs=1) as cp_pool,
        ):
            w_sb = w_pool.tile([128, KT, F], bf16)
            nc.sync.dma_start(out=w_sb[:], in_=w_t[:])

            cp_acc = cp_pool.tile([128, B_CORE], fp32)

            for g in range(N_GROUPS):
                b0 = g * BG

                # replicated x0 tiles: [p=(c_l, n_l), (b_l, d)]
                # row p gets x0[b, 16ni + p%16, d], same for all 8 c_l
                x0_reps = []
                for ni in range(NNI):
                    t = x0_pool.tile([128, COLS], bf16, tag="x0rep")
                    src = x0_r[16 * ni : 16 * ni + NB, b0 : b0 + BG, :]
                    bsl = src.unsqueeze(0).broadcast_to((CA, NB, BG, D))
                    nc.scalar.dma_start(out=t[:], in_=bsl)
                    x0_reps.append(t)

                ps = [
                    ps_pool.tile([128, QTR], fp32, tag=f"ps{j}", name=f"ps{j}_{g}")
                    for j in range(4)
                ]

                for ci in range(NCI):
                    # replicated xj tile: row p gets xj[b, 8ci + p//16, d],
                    # same for all 16 n_l; freed after its 4 k-tiles
                    xj_rep = xj_pool.tile([128, COLS], bf16, tag="xjrep")
                    src = xj_r[8 * ci : 8 * ci + 8, b0 : b0 + BG, :]
                    bsl = src.unsqueeze(1).broadcast_to((CA, NB, BG, D))
                    nc.sync.dma_start(out=xj_rep[:], in_=bsl)

                    for ni in range(NNI):
                        kt = ci * NNI + ni
                        h = h_pool.tile([128, COLS], bf16)
                        # offload 1 of 8 Hadamard muls to the otherwise-idle
                        # GpSimd engine (~2x slower per element, separate unit)
                        eng = nc.gpsimd if (kt % 8 == 3) else nc.vector
                        eng.tensor_mul(h[:], xj_rep[:], x0_reps[ni][:])
                        for j in range(4):
                            nc.tensor.matmul(
                                ps[j][:],
                                w_sb[:, kt, :],
                                h[:, j * QTR : (j + 1) * QTR],
                                start=(kt == 0),
                                stop=(kt == KT - 1),
                            )

                # evacuate PSUM -> SBUF (fp32) on the Scalar engine
                co = co_pool.tile([128, COLS], fp32)
                for j in range(4):
                    nc.scalar.copy(co[:, j * QTR : (j + 1) * QTR], ps[j][:])

                # cin_p partial: sum over d (innermost 64) per (f, b)
                for j in range(4):
                    nc.vector.tensor_reduce(
                        out=cp_acc[:, g * BG + j * 8 : g * BG + (j + 1) * 8],
                        in_=ps[j][:].rearrange("p (b d) -> p b d", d=D),
                        axis=mybir.AxisListType.X,
                        op=mybir.AluOpType.add,
                    )

                # store cin_out rows b0..b0+BG: dest viewed (f, b_l, d)
                nc.scalar.dma_start(
                    out=cout[b0 : b0 + BG].rearrange("b f d -> f b d"),
                    in_=co[:].rearrange("p (b d) -> p b d", d=D),
                )

            nc.scalar.dma_start(out=cp_t[:], in_=cp_acc[:])

    nc.compile()
    return nc


def kernel(xj: np.ndarray, x0: np.ndarray, W: np.ndarray, trace: bool = False):
    global _compiled, LAST_RESULTS
    from concourse.bass_utils import run_bass_kernel_spmd

    if _compiled is None:
        _compiled = _build()
    nc = _compiled

    xj_r = np.ascontiguousarray(np.transpose(np.asarray(xj), (1, 0, 2))).astype(BF16)
    x0_r = np.ascontiguousarray(np.transpose(np.asarray(x0), (1, 0, 2))).astype(BF16)
    # w_t[p=(c_l,n_l), kt=(ci,ni), f] = W[f, 8ci+c_l, 16ni+n_l]
    W2 = np.transpose(np.asarray(W), (1, 2, 0))            # (C, N, F)
    W3 = W2.reshape(NCI, CA, NNI, NB, F)                   # ci, c_l, ni, n_l, f
    w_t = np.ascontiguousarray(
        np.transpose(W3, (1, 3, 0, 2, 4)).reshape(128, KT, F)
    ).astype(BF16)

    in_maps = []
    for i in range(N_CORES):
        s = slice(i * B_CORE, (i + 1) * B_CORE)
        in_maps.append(
            {
                "xj_r": np.ascontiguousarray(xj_r[:, s, :]),
                "x0_r": np.ascontiguousarray(x0_r[:, s, :]),
                "w_t": w_t,
            }
        )

    res = run_bass_kernel_spmd(nc, in_maps, list(range(N_CORES)), trace=trace)
    LAST_RESULTS = res

    cin_out = np.concatenate([res.results[i]["cout"] for i in range(N_CORES)], axis=0)
    cin_p = np.concatenate(
        [res.results[i]["cp_t"].T for i in range(N_CORES)], axis=0
    )
    return cin_out, cin_p


# revision 12
# speedup vs baseline: 1.2206x; 1.2206x over previous
"""Trainium2 Bass kernel for nn_CINLayer.

Computes, for B=2048, C=64, N=64, D=64, F=128:
    cin_out[b,f,d] = sum_{c,n} W[f,c,n] * xj[b,c,d] * x0[b,n,d]   (B, F, D)
    cin_p_out[b,f] = sum_d cin_out[b,f,d]                          (B, F)

Strategy (per NeuronCore, data-parallel over b across 8 cores):
  The einsum is reassociated as one accumulated matmul over K=(c,n)=4096:
     cin_out[f, (b,d)] = W_flat[f, (c,n)] @ H[(c,n), (b,d)]
  with H[(c,n),(b,d)] = xj[b,c,d]*x0[b,n,d] built on the Vector engine in
  bf16 (2x packed mode) one 128-row k-tile at a time.

  A k-tile (ci, ni) covers c in [8ci, 8ci+8) x n in [16ni, 16ni+16), so the
  partition-broadcast operands factorize: H(ci,ni) = xj_rep(ci) * x0_rep(ni),
  and only 8 + 4 = 12 replicated (128, 2048) tiles are DMA'd per batch group
  of 32 rows instead of 32 tiles (each xj_rep is reused for 4 k-tiles, each
  x0_rep for 8) — the minimum replication traffic for materialized operands.
  The replication DMA sources are host-pre-transposed (C,B,D)/(N,B,D) bf16
  copies so every replicated partition row is one contiguous 4KB read.
  The 32 k-tile matmuls (bf16, full PE rate, Nf=512 into 4 PSUM banks)
  accumulate in PSUM; ScalarE evacuates to SBUF; cin_p comes from a
  free-dim tensor_reduce of each PSUM bank.

  Measured on hardware: 376.6 us end-to-end (8 cores), rel-l2 err 3.3e-3.
  Bottleneck: the 8 load-side SDMA engines moving the ~46MB/core of
  replicated operands (~17.5 GB/s each); PE matmul stream ~276 us,
  DVE Hadamard ~313 us, all overlapped.
"""

import sys

for _p in ("/opt/trn_rl_repo", "/root/.axon_site/_ro/trn_rl_repo"):
    if _p not in sys.path:
        sys.path.insert(0, _p)

import numpy as np
import ml_dtypes

BF16 = ml_dtypes.bfloat16

B, C, N, D, F = 2048, 64, 64, 64, 128
N_CORES = 8
B_CORE = B // N_CORES          # 256 batch rows per core
BG = 32                        # batch rows per group (=> 2048 matmul columns)
N_GROUPS = B_CORE // BG        # 16 groups
KT = 32                        # number of 128-row k-tiles (C*N/128)
COLS = BG * D                  # 1024 columns per group
QTR = COLS // 4                # 512 = one PSUM bank of fp32
CA = 8                         # c's per k-tile
NB = 128 // CA                 # n's per k-tile (16)
NCI = C // CA                  # 8 ci tiles per group
NNI = N // NB                  # 4 ni tiles per group

_compiled = None
LAST_RESULTS = None


def _build():
    import concourse.bass as bass
    import concourse.mybir as mybir
    import concourse.tile as tile
    from concourse import bacc

    fp32 = mybir.dt.float32
    bf16 = mybir.dt.bfloat16

    nc = bacc.Bacc("TRN2", target_bir_lowering=False, debug=False)

    xj_r = nc.dram_tensor("xj_r", [C, B_CORE, D], bf16, kind="ExternalInput").ap()
    x0_r = nc.dram_tensor("x0_r", [N, B_CORE, D], bf16, kind="ExternalInput").ap()
    w_t = nc.dram_tensor("w_t", [128, KT, F], bf16, kind="ExternalInput").ap()
    cout = nc.dram_tensor("cout", [B_CORE, F, D], fp32, kind="ExternalOutput").ap()
    cp_t = nc.dram_tensor("cp_t", [F, B_CORE], fp32, kind="ExternalOutput").ap()

    with tile.TileContext(nc) as tc:
        with (
            tc.tile_pool(name="w", bufs=1) as w_pool,
            tc.tile_pool(name="xjrep", bufs=16) as xj_pool,
            tc.tile_pool(name="x0rep", bufs=8) as x0_pool,
            tc.tile_pool(name="h", bufs=8) as h_pool,
            tc.tile_pool(name="ps", bufs=2, space="PSUM") as ps_pool,
            tc.tile_pool(name="co", bufs=2) as co_pool,
            tc.tile_pool(name="cp", bufs=1) as cp_pool,
        ):
            w_sb = w_pool.tile([128, KT, F], bf16)
            nc.sync.dma_start(out=w_sb[:], in_=w_t[:])

            cp_acc = cp_pool.tile([128, B_CORE], fp32)

            for g in range(N_GROUPS):
                b0 = g * BG

                # replicated x0 tiles: [p=(c_l, n_l), (b_l, d)]
                # row p gets x0[b, 16ni + p%16, d], same for all 8 c_l
                x0_reps = []
                for ni in range(NNI):
                    t = x0_pool.tile([128, COLS], bf16, tag="x0rep")
                    src = x0_r[16 * ni : 16 * ni + NB, b0 : b0 + BG, :]
                    bsl = src.unsqueeze(0).broadcast_to((CA, NB, BG, D))
                    nc.gpsimd.dma_start(out=t[:], in_=bsl)
                    x0_reps.append(t)

                ps = [
                    ps_pool.tile([128, QTR], fp32, tag=f"ps{j}", name=f"ps{j}_{g}")
                    for j in range(4)
                ]

                for ci in range(NCI):
                    # replicated xj tile: row p gets xj[b, 8ci + p//16, d],
                    # same for all 16 n_l; freed after its 4 k-tiles
                    xj_rep = xj_pool.tile([128, COLS], bf16, tag="xjrep")
                    src = xj_r[8 * ci : 8 * ci + 8, b0 : b0 + BG, :]
                    bsl = src.unsqueeze(1).broadcast_to((CA, NB, BG, D))
                    nc.sync.dma_start(out=xj_rep[:], in_=bsl)

                    for ni in range(NNI):
                        kt = ci * NNI + ni
                        h = h_pool.tile([128, COLS], bf16)
                        nc.vector.tensor_mul(h[:], xj_rep[:], x0_reps[ni][:])
                        for j in range(4):
                            nc.tensor.matmul(
                                ps[j][:],
                                w_sb[:, kt, :],
                                h[:, j * QTR : (j + 1) * QTR],
                                start=(kt == 0),
                                stop=(kt == KT - 1),
                            )

                # evacuate PSUM -> SBUF (fp32) on the Scalar engine
                co = co_pool.tile([128, COLS], fp32)
                for j in range(4):
                    nc.scalar.copy(co[:, j * QTR : (j + 1) * QTR], ps[j][:])

                # cin_p partial: sum over d (innermost 64) per (f, b)
                for j in range(4):
                    nc.vector.tensor_reduce(
                        out=cp_acc[:, g * BG + j * 8 : g * BG + (j + 1) * 8],
                        in_=ps[j][:].rearrange("p (b d) -> p b d", d=D),
                        axis=mybir.AxisListType.X,
                        op=mybir.AluOpType.add,
                    )

                # store cin_out rows b0..b0+BG: dest viewed (f, b_l, d)
                nc.scalar.dma_start(
                    out=cout[b0 : b0 + BG].rearrange("b f d -> f b d"),
                    in_=co[:].rearrange("p (b d) -> p b d", d=D),
                )

            nc.scalar.dma_start(out=cp_t[:], in_=cp_acc[:])

    nc.compile()
    return nc


def kernel(xj: np.ndarray, x0: np.ndarray, W: np.ndarray, trace: bool = False):
    global _compiled, LAST_RESULTS
    from concourse.bass_utils import run_bass_kernel_spmd

    if _compiled is None:
        _compiled = _build()
    nc = _compiled

    xj_r = np.ascontiguousarray(np.transpose(np.asarray(xj), (1, 0, 2))).astype(BF16)
    x0_r = np.ascontiguousarray(np.transpose(np.asarray(x0), (1, 0, 2))).astype(BF16)
    # w_t[p=(c_l,n_l), kt=(ci,ni), f] = W[f, 8ci+c_l, 16ni+n_l]
    W2 = np.transpose(np.asarray(W), (1, 2, 0))            # (C, N, F)
    W3 = W2.reshape(NCI, CA, NNI, NB, F)                   # ci, c_l, ni, n_l, f
    w_t = np.ascontiguousarray(
        np.transpose(W3, (1, 3, 0, 2, 4)).reshape(128, KT, F)
    ).astype(BF16)

    in_maps = []
    for i in range(N_CORES):
        s = slice(i * B_CORE, (i + 1) * B_CORE)
        in_maps.append(
            {
                "xj_r": np.ascontiguousarray(xj_r[:, s, :]),
                "x0_r": np.ascontiguousarray(x0_r[:, s, :]),
                "w_t": w_t,
            }
        )

    res = run_bass_kernel_spmd(nc, in_maps, list(range(N_CORES)), trace=trace)
    LAST_RESULTS = res

    cin_out = np.concatenate([res.results[i]["cout"] for i in range(N_CORES)], axis=0)
    cin_p = np.concatenate(
        [res.results[i]["cp_t"].T for i in range(N_CORES)], axis=0
    )
    return cin_out, cin_p


# revision 13
# speedup vs baseline: 1.2805x; 1.0491x over previous
"""Trainium2 Bass kernel for nn_CINLayer.

Computes, for B=2048, C=64, N=64, D=64, F=128:
    cin_out[b,f,d] = sum_{c,n} W[f,c,n] * xj[b,c,d] * x0[b,n,d]   (B, F, D)
    cin_p_out[b,f] = sum_d cin_out[b,f,d]                          (B, F)

Strategy (per NeuronCore, data-parallel over b across 8 cores):
  The einsum is reassociated as one accumulated matmul over K=(c,n)=4096:
     cin_out[f, (b,d)] = W_flat[f, (c,n)] @ H[(c,n), (b,d)]
  with H[(c,n),(b,d)] = xj[b,c,d]*x0[b,n,d] built on the Vector engine in
  bf16 (2x packed mode) one 128-row k-tile at a time.

  A k-tile (ci, ni) covers c in [8ci, 8ci+8) x n in [16ni, 16ni+16), so the
  partition-broadcast operands factorize: H(ci,ni) = xj_rep(ci) * x0_rep(ni),
  and only 8 + 4 = 12 replicated (128, 2048) tiles are DMA'd per batch group
  of 32 rows instead of 32 tiles (each xj_rep is reused for 4 k-tiles, each
  x0_rep for 8) — the minimum replication traffic for materialized operands.
  The replication DMA sources are host-pre-transposed (C,B,D)/(N,B,D) bf16
  copies so every replicated partition row is one contiguous 4KB read.
  The 32 k-tile matmuls (bf16, full PE rate, Nf=512 into 4 PSUM banks)
  accumulate in PSUM; ScalarE evacuates to SBUF; cin_p comes from a
  free-dim tensor_reduce of each PSUM bank.

  Measured on hardware: 376.6 us end-to-end (8 cores), rel-l2 err 3.3e-3.
  Bottleneck: the 8 load-side SDMA engines moving the ~46MB/core of
  replicated operands (~17.5 GB/s each); PE matmul stream ~276 us,
  DVE Hadamard ~313 us, all overlapped.
"""

import sys

for _p in ("/opt/trn_rl_repo", "/root/.axon_site/_ro/trn_rl_repo"):
    if _p not in sys.path:
        sys.path.insert(0, _p)

import numpy as np
import ml_dtypes

BF16 = ml_dtypes.bfloat16

B, C, N, D, F = 2048, 64, 64, 64, 128
N_CORES = 8
B_CORE = B // N_CORES          # 256 batch rows per core
BG = 32                        # batch rows per group (=> 2048 matmul columns)
N_GROUPS = B_CORE // BG        # 16 groups
KT = 32                        # number of 128-row k-tiles (C*N/128)
COLS = BG * D                  # 1024 columns per group
QTR = COLS // 4                # 512 = one PSUM bank of fp32
CA = 8                         # c's per k-tile
NB = 128 // CA                 # n's per k-tile (16)
NCI = C // CA                  # 8 ci tiles per group
NNI = N // NB                  # 4 ni tiles per group

_compiled = None
LAST_RESULTS = None


def _build():
    import concourse.bass as bass
    import concourse.mybir as mybir
    import concourse.tile as tile
    from concourse import bacc

    fp32 = mybir.dt.float32
    bf16 = mybir.dt.bfloat16

    nc = bacc.Bacc("TRN2", target_bir_lowering=False, debug=False)

    xj_r = nc.dram_tensor("xj_r", [C, B_CORE, D], bf16, kind="ExternalInput").ap()
    x0_r = nc.dram_tensor("x0_r", [N, B_CORE, D], bf16, kind="ExternalInput").ap()
    w_t = nc.dram_tensor("w_t", [128, KT, F], bf16, kind="ExternalInput").ap()
    zb_d = nc.dram_tensor("zb", [64, NNI * 128], bf16, kind="ExternalInput").ap()
    cout = nc.dram_tensor("cout", [B_CORE, F, D], fp32, kind="ExternalOutput").ap()
    cp_t = nc.dram_tensor("cp_t", [F, B_CORE], fp32, kind="ExternalOutput").ap()

    with tile.TileContext(nc) as tc:
        with (
            tc.tile_pool(name="w", bufs=1) as w_pool,
            tc.tile_pool(name="xjrep", bufs=16) as xj_pool,
            tc.tile_pool(name="x0rep", bufs=8) as x0_pool,
            tc.tile_pool(name="h", bufs=8) as h_pool,
            tc.tile_pool(name="ps", bufs=1, space="PSUM") as ps_pool,
            tc.tile_pool(name="psbc", bufs=3, space="PSUM") as psbc_pool,
            tc.tile_pool(name="x0c", bufs=2) as x0c_pool,
            tc.tile_pool(name="co", bufs=2) as co_pool,
            tc.tile_pool(name="cp", bufs=1) as cp_pool,
        ):
            w_sb = w_pool.tile([128, KT, F], bf16)
            nc.sync.dma_start(out=w_sb[:], in_=w_t[:])
            zb_sb = w_pool.tile([64, NNI * 128], bf16)
            nc.sync.dma_start(out=zb_sb[:], in_=zb_d[:])

            cp_acc = cp_pool.tile([128, B_CORE], fp32)

            for g in range(N_GROUPS):
                b0 = g * BG

                # compact x0 slice: (64 n-partitions, (b_l, d))
                x0c = x0c_pool.tile([64, COLS], bf16)
                nc.sync.dma_start(out=x0c[:], in_=x0_r[:, b0 : b0 + BG, :])

                # replicated x0 tiles built on the PE: selection matmul
                # out[p=(c_l,n_l), col] = x0c[16ni + n_l, col], then ScalarE
                # evacuates PSUM to SBUF as bf16. Saves 16MB of load-side DMA.
                x0_reps = []
                for ni in range(NNI):
                    t = x0_pool.tile([128, COLS], bf16, tag="x0rep")
                    for j in range(4):
                        pbc = psbc_pool.tile([128, QTR], fp32, tag="psbc",
                                             name=f"psbc_{g}_{ni}_{j}")
                        nc.tensor.matmul(
                            pbc[:],
                            zb_sb[:, ni * 128 : (ni + 1) * 128],
                            x0c[:, j * QTR : (j + 1) * QTR],
                            start=True,
                            stop=True,
                        )
                        nc.scalar.copy(t[:, j * QTR : (j + 1) * QTR], pbc[:])
                    x0_reps.append(t)

                ps = [
                    ps_pool.tile([128, QTR], fp32, tag=f"ps{j}", name=f"ps{j}_{g}")
                    for j in range(4)
                ]

                for ci in range(NCI):
                    # replicated xj tile: row p gets xj[b, 8ci + p//16, d],
                    # same for all 16 n_l; freed after its 4 k-tiles
                    xj_rep = xj_pool.tile([128, COLS], bf16, tag="xjrep")
                    src = xj_r[8 * ci : 8 * ci + 8, b0 : b0 + BG, :]
                    bsl = src.unsqueeze(1).broadcast_to((CA, NB, BG, D))
                    nc.sync.dma_start(out=xj_rep[:], in_=bsl)

                    for ni in range(NNI):
                        kt = ci * NNI + ni
                        h = h_pool.tile([128, COLS], bf16)
                        nc.vector.tensor_mul(h[:], xj_rep[:], x0_reps[ni][:])
                        for j in range(4):
                            nc.tensor.matmul(
                                ps[j][:],
                                w_sb[:, kt, :],
                                h[:, j * QTR : (j + 1) * QTR],
                                start=(kt == 0),
                                stop=(kt == KT - 1),
                            )

                # evacuate PSUM -> SBUF (fp32) on the Scalar engine
                co = co_pool.tile([128, COLS], fp32)
                for j in range(4):
                    nc.scalar.copy(co[:, j * QTR : (j + 1) * QTR], ps[j][:])

                # cin_p partial: sum over d (innermost 64) per (f, b)
                for j in range(4):
                    nc.vector.tensor_reduce(
                        out=cp_acc[:, g * BG + j * 8 : g * BG + (j + 1) * 8],
                        in_=ps[j][:].rearrange("p (b d) -> p b d", d=D),
                        axis=mybir.AxisListType.X,
                        op=mybir.AluOpType.add,
                    )

                # store cin_out rows b0..b0+BG: dest viewed (f, b_l, d)
                nc.scalar.dma_start(
                    out=cout[b0 : b0 + BG].rearrange("b f d -> f b d"),
                    in_=co[:].rearrange("p (b d) -> p b d", d=D),
                )

            nc.scalar.dma_start(out=cp_t[:], in_=cp_acc[:])

    nc.compile()
    return nc


def kernel(xj: np.ndarray, x0: np.ndarray, W: np.ndarray, trace: bool = False):
    global _compiled, LAST_RESULTS
    from concourse.bass_utils import run_bass_kernel_spmd

    if _compiled is None:
        _compiled = _build()
    nc = _compiled

    xj_r = np.ascontiguousarray(np.transpose(np.asarray(xj), (1, 0, 2))).astype(BF16)
    x0_r = np.ascontiguousarray(np.transpose(np.asarray(x0), (1, 0, 2))).astype(BF16)
    # w_t[p=(c_l,n_l), kt=(ci,ni), f] = W[f, 8ci+c_l, 16ni+n_l]
    W2 = np.transpose(np.asarray(W), (1, 2, 0))            # (C, N, F)
    W3 = W2.reshape(NCI, CA, NNI, NB, F)                   # ci, c_l, ni, n_l, f
    w_t = np.ascontiguousarray(
        np.transpose(W3, (1, 3, 0, 2, 4)).reshape(128, KT, F)
    ).astype(BF16)
    zb = np.zeros((64, NNI, 128), np.float32)
    for ni in range(NNI):
        for p in range(128):
            zb[16 * ni + p % 16, ni, p] = 1.0
    zb = np.ascontiguousarray(zb.reshape(64, NNI * 128)).astype(BF16)

    in_maps = []
    for i in range(N_CORES):
        s = slice(i * B_CORE, (i + 1) * B_CORE)
        in_maps.append(
            {
                "xj_r": np.ascontiguousarray(xj_r[:, s, :]),
                "x0_r": np.ascontiguousarray(x0_r[:, s, :]),
                "w_t": w_t,
                "zb": zb,
            }
        )

    res = run_bass_kernel_spmd(nc, in_maps, list(range(N_CORES)), trace=trace)
    LAST_RESULTS = res

    cin_out = np.concatenate([res.results[i]["cout"] for i in range(N_CORES)], axis=0)
    cin_p = np.concatenate(
        [res.results[i]["cp_t"].T for i in range(N_CORES)], axis=0
    )
    return cin_out, cin_p
